# revision 68
# baseline (speedup 1.0000x reference)
"""Multi-head attention (B=8, N=1024, H=12, D=64, C=768) on 8 trn2 cores.

Sharding: data-parallel over batch. Core b computes attention for x[b];
weights are replicated. No collectives.

v3 dataflow (fp16 matmul operands, fp32 PSUM accumulate):
  phase 1a: qkT[1536 x N] = W_qkv[:, :1536].T @ x^T    (d-major Q^T, K^T)
  phase 1b: v_aug[N x H x 65] = x @ W_qkv[:, 1536:]    (+ ones column)
  phase 2 (per head pair t=(2t,2t+1), per n-half nh, per m-tile):
     S^T[m,n] = k^T.T @ q^T                (2x K=64 matmuls, ap=512)
     P^T = exp(S^T / 8) -> fp16            (ScalarE, one [128,1024] op)
     acc[n,(j,nb),0:65] += P^T-block.T @ v_aug   (flipped PV: stationary
        = P^T [128m,128n] block, moving = v_aug [128m,65]; all 128
        n-partitions live -> 2.4x fewer PE cycles than v-stationary.
        col 64 accumulates the softmax denominator via the ones column.)
     normalize: h[n,d] = acc * (1/acc[:,:,64])  (DVE tensor_scalar)
     transpose: hT[d,n] via DMA xbar transpose (no PE/DVE cost;
        PE transpose for the final group, which gates the proj tail)
  phase 3: y = hT.T @ W_proj
Scheduling: the Tile list scheduler is greedy by priority among READY
instructions; the S/exp stream runs in a top-priority band (ScalarE's
96 x 1.04us exp chain is the pacing stream), PVs and normalize in
middle bands, and qk/v/proj fill PE slack at natural priorities.
Pair-0 Q/K weight columns are DMA'd first so the exp stream starts as
soon as x lands; dummy matmuls ramp the PE p-state to max clock while
DMAs land. PSUM note: a start=True matmul zeroes its whole 2KB bank
row, so only the first PV block per bank carries start=True.
"""
from contextlib import nullcontext

import numpy as np

import concourse.bass as bass
import concourse.mybir as mybir
import concourse.tile as tile
from concourse import bacc
from concourse.bass_utils import run_bass_kernel_spmd
from concourse.masks import make_identity

F32 = mybir.dt.float32
F16 = mybir.dt.float16

B, N, C = 8, 1024, 768
H, D = 12, 64
HID = H * D  # 768
KT = C // 128          # 6 feature k-tiles
MT = N // 128          # 8 sequence m-tiles
SCALE = D ** -0.5      # 0.125

_cached_nc = None

DEFAULT_OPTS = dict(
    s_bufs=2, acc_bufs=1, mm1_bufs=2, pt_bufs=44, hoist_pair0=True,
)


def build_program(**opts):
    o = dict(DEFAULT_OPTS, **opts)
    nc = bacc.Bacc(None, target_bir_lowering=False)

    xT_d = nc.dram_tensor("xT", [C, N], F16, kind="ExternalInput")
    wqkv_d = nc.dram_tensor("wqkv", [C, 3 * HID], F16, kind="ExternalInput")
    wproj_d = nc.dram_tensor("wproj", [HID, C], F16, kind="ExternalInput")
    y_d = nc.dram_tensor("y", [N, C], F16, kind="ExternalOutput")

    with tile.TileContext(nc) as tc:
        with tc.tile_pool(name="persist", bufs=1) as persist, \
             tc.tile_pool(name="pt_pool", bufs=o["pt_bufs"]) as pt_pool, \
             tc.tile_pool(name="nrm_pool", bufs=6) as nrm_pool, \
             tc.tile_pool(name="y_pool", bufs=6) as y_pool, \
             tc.tile_pool(name="ps_a", bufs=o["mm1_bufs"], space="PSUM") as ps_a, \
             tc.tile_pool(name="ps_s", bufs=o["s_bufs"], space="PSUM") as ps_s, \
             tc.tile_pool(name="ps_acc", bufs=o["acc_bufs"], space="PSUM") as ps_acc:

            # ---- resident tiles (merged k-dim: fewer DMAs; the single
            # HWDGE device serializes descriptor generation at ~625ns per
            # DMA, so DMA count gates how fast inputs land) ----
            xt_t = persist.tile([128, KT, N], F16, name="xt", tag="xt")
            wqk06_t = persist.tile([128, 2, KT, 128], F16, name="wqk06",
                                   tag="wqk06")
            wqk_t = persist.tile([128, KT, 2 * HID], F16, name="wqk", tag="wqk")
            wv_t = persist.tile([128, KT, HID], F16, name="wv", tag="wv")
            wp_t = persist.tile([128, KT, C], F16, name="wp", tag="wp")
            xt = [xt_t[:, k, :] for k in range(KT)]
            wqk = [wqk_t[:, k, :] for k in range(KT)]
            wv = [wv_t[:, k, :] for k in range(KT)]
            wp = [wp_t[:, k, :] for k in range(KT)]

            # DMA priority: pair-0 qk weight cols + x first (feeds the
            # first two qk tiles and thus the exp stream), then v weights,
            # remaining qk weights, proj weights.
            xT_r = xT_d.rearrange("(k p) n -> p k n", p=128)
            wqkv_r = wqkv_d.rearrange("(k p) c -> p k c", p=128)
            wproj_r = wproj_d.rearrange("(k p) c -> p k c", p=128)
            # x lands column-split: the first 512 n-cols feed the qkT
            # nh0 halves (and S m-tiles 0..3), so the exp stream starts
            # after only half of x has crossed the serial DMA engine.
            nc.sync.dma_start(wqk06_t[:, 0], wqkv_r[:, :, 0:128])
            nc.sync.dma_start(wqk06_t[:, 1], wqkv_r[:, :, HID:HID + 128])
            nc.sync.dma_start(xt_t[:, :, 0:512], xT_r[:, :, 0:512])
            nc.sync.dma_start(xt_t[:, :, 512:], xT_r[:, :, 512:])
            for i in range(2):
                nc.sync.dma_start(wv_t[:, 3 * i:3 * i + 3, :],
                                  wqkv_r[:, 3 * i:3 * i + 3, 2 * HID:])
            for i in range(3):
                nc.sync.dma_start(wqk_t[:, 2 * i:2 * i + 2, :],
                                  wqkv_r[:, 2 * i:2 * i + 2, :2 * HID])
            for i in range(2):
                nc.sync.dma_start(wp_t[:, 3 * i:3 * i + 3, :],
                                  wproj_r[:, 3 * i:3 * i + 3, :])

            # warm the exp table set during the DMA prefix (the ACT
            # table load otherwise lands on the first real exp)
            warm = persist.tile([1, 8], F32, name="warm", tag="warm")
            nc.gpsimd.memset(warm[:], 0.0)
            nc.scalar.activation(warm[:], warm[:],
                                 mybir.ActivationFunctionType.Exp)

            # identity for the last-group PE transpose
            ident = persist.tile([128, 128], F16, name="ident", tag="ident")
            make_identity(nc, ident[:])

            # PE p-state ramp: ~3us of back-to-back dummy matmuls while the
            # first DMAs land, so real matmuls start at max clock instead
            # of paying the 0.65/1.2 GHz warm-up on the critical path.
            junk = persist.tile([128, 128], F16, name="junk", tag="junk")
            nc.gpsimd.memset(junk[:], 0.0)
            ps_j = ps_a.tile([128, 128], F32, name="ps_junk", tag="mm1")
            for _ in range(40):
                nc.tensor.matmul(ps_j[:], junk[:], junk[:],
                                 start=True, stop=True)

            qkT = [persist.tile([128, N], F16, name=f"qkT{t}", tag=f"qkT{t}")
                   for t in range(12)]
            v_aug = [persist.tile([128, H, D + 1], F16, name=f"vaug{m}",
                                  tag=f"vaug{m}")
                     for m in range(MT)]
            hT = [persist.tile([128, N], F16, name=f"hT{t}", tag=f"hT{t}")
                  for t in range(KT)]

            # ---- phase 1a: one qkT tile (output rows = qkv cols t*128..) ----
            def qk_tile(t, halves=(0, 1)):
                for nhalf in halves:
                    ps = ps_a.tile([128, 512], F32, name="ps_qk", tag="mm1")
                    for k in range(KT):
                        if t == 0 or t == 6:
                            w = wqk06_t[:, 0 if t == 0 else 1, k, :]
                        else:
                            w = wqk[k][:, t * 128:(t + 1) * 128]
                        nc.tensor.matmul(ps[:], w,
                                         xt[k][:, nhalf * 512:(nhalf + 1) * 512],
                                         start=(k == 0), stop=(k == KT - 1))
                    nc.vector.tensor_copy(qkT[t][:, nhalf * 512:(nhalf + 1) * 512],
                                          ps[:])

            # ---- phase 1b: v tiles ----
            def v_tile(m):
                for vh in range(2):
                    ps = ps_a.tile([128, 384], F32, name="ps_v", tag="mm1")
                    for k in range(KT):
                        nc.tensor.matmul(ps[:], xt[k][:, m * 128:(m + 1) * 128],
                                         wv[k][:, vh * 384:(vh + 1) * 384],
                                         start=(k == 0), stop=(k == KT - 1))
                    dst = v_aug[m][:, vh * 6:(vh + 1) * 6, 0:D]
                    nc.vector.tensor_copy(dst,
                                          ps[:].rearrange("p (h d) -> p h d", d=D))
                nc.gpsimd.memset(v_aug[m][:, :, D:D + 1], 1.0)

            # Priority bands inside the attention stream: S+exp run at
            # absolute top priority (they feed ScalarE, the pacing engine);
            # normalize at ~5 and PVs at ~10 so a post-boundary PV backlog
            # can never delay the next S matmul; background (qk/v/proj)
            # keeps natural emission priorities (~100+).
            def band(prio):
                return tc.high_priority(offset=tc.cur_priority - prio)

            # ---- phase 2: attention for head pair (2t, 2t+1), n-half nh ----
            def attention_nh(t, nh, last=False):
                qT_t, kT_t = qkT[t], qkT[6 + t]
                nsl = slice(nh * 512, (nh + 1) * 512)
                acc = ps_acc.tile([128, 8, 128], F32, name="acc", tag="acc")
                for m in range(MT):
                    msl = slice(m * 128, (m + 1) * 128)
                    with tc.high_priority():
                        s_ps = ps_s.tile([128, 1024], F32, name="s_ps", tag="s")
                        for j in range(2):
                            psl = slice(j * 64, (j + 1) * 64)
                            nc.tensor.matmul(s_ps[:, j * 512:(j + 1) * 512],
                                             kT_t[psl, msl], qT_t[psl, nsl],
                                             start=True, stop=True)
                        p_sb = pt_pool.tile([128, 1024], F16, name="p_sb", tag="p")
                        nc.scalar.activation(p_sb[:], s_ps[:],
                                             mybir.ActivationFunctionType.Exp,
                                             scale=SCALE)
                    with band(10):
                        for j in range(2):
                            for nb in range(4):
                                nc.tensor.matmul(
                                    acc[:, j * 4 + nb, 0:D + 1],
                                    p_sb[:, j * 512 + nb * 128:j * 512 + (nb + 1) * 128],
                                    v_aug[m][:, 2 * t + j, :],
                                    start=(m == 0 and nb == 0),
                                    stop=(m == MT - 1))
                # normalize (DVE per-partition scalar), then transpose into
                # hT: DMA xbar transpose (no PE/DVE cost) except for the
                # last group, where PE-transpose latency is lower and the
                # final proj tiles are gated on it.
                with band(5):
                    rs = nrm_pool.tile([128, 8], F32, name="rs", tag="rs")
                    nc.vector.reciprocal(rs[:], acc[:, :, D])
                    tp4 = (ps_a.tile([128, 4, 128], F16, name="tp4", tag="mm1")
                           if last else None)
                    for nb in range(4):
                        hst = nrm_pool.tile([128, 128], F16, name="hst", tag="hst")
                        for j in range(2):
                            nc.vector.tensor_scalar_mul(
                                hst[:, j * D:(j + 1) * D],
                                acc[:, j * 4 + nb, 0:D],
                                rs[:, j * 4 + nb:j * 4 + nb + 1])
                        csl = slice(nh * 512 + nb * 128, nh * 512 + (nb + 1) * 128)
                        if last:
                            # PE transposes batched in ONE psum allocation (a
                            # single ps_a rotation slot -> the tail proj
                            # groups can pre-start k<5 matmuls during this
                            # group), with per-quadrant evicts so each hT
                            # block lands as soon as its transpose is done.
                            # start=True only on nb==0: a start zeroes the
                            # whole 2KB bank row; later blocks replace.
                            nc.tensor.matmul(tp4[:, nb, :], hst[:], ident[:],
                                             is_transpose=True,
                                             start=(nb == 0), stop=(nb == 3))
                            nc.scalar.copy(hT[t][:, csl], tp4[:, nb, :])
                        else:
                            nc.sync.dma_start_transpose(hT[t][:, csl], hst[:])

            # ---- phase 3: y = hT.T @ W_proj ----
            def proj(m, tail=False):
                for ph in range(2):
                    ps = ps_a.tile([128, 384], F32, name="ps_y", tag="mm1")
                    for k in range(KT):
                        nc.tensor.matmul(ps[:], hT[k][:, m * 128:(m + 1) * 128],
                                         wp[k][:, ph * 384:(ph + 1) * 384],
                                         start=(k == 0), stop=(k == KT - 1))
                    y_sb = y_pool.tile([128, 384], F16, name="y_sb", tag="y")
                    nc.vector.tensor_copy(y_sb[:], ps[:])
                    yq = nc.sync if (2 * m + ph) % 2 == 0 else nc.scalar
                    yq.dma_start(
                        y_d[m * 128:(m + 1) * 128, ph * 384:(ph + 1) * 384],
                        y_sb[:])

            # ---- schedule ----
            # Emission order = dataflow order (tile deps derive from it);
            # the scheduler's greedy choice is steered by the bands above.
            qk_tile(0, halves=(0,))
            qk_tile(6, halves=(0,))
            qk_tile(0, halves=(1,))
            qk_tile(6, halves=(1,))
            qk_tile(1)
            qk_tile(7)
            for m in range(MT):
                v_tile(m)
            attention_nh(0, 0)
            for t in range(1, 6):
                if t < 5:
                    qk_tile(t + 1)
                    qk_tile(6 + t + 1)
                attention_nh(t, 0)
            for m in range(4):
                proj(m)
            for t in range(6):
                attention_nh(t, 1, last=(t == 5))
            for m in range(4, MT):
                proj(m, tail=True)

    nc.compile()
    return nc


def _run(inputs, trace=False, trace_kwargs=None):
    global _cached_nc
    x = np.asarray(inputs["x"], dtype=np.float32)
    wqkv = np.ascontiguousarray(
        np.asarray(inputs["W_qkv"], dtype=np.float32)).astype(np.float16)
    wproj = np.ascontiguousarray(
        np.asarray(inputs["W_proj"], dtype=np.float32)).astype(np.float16)
    xT = np.ascontiguousarray(x.transpose(0, 2, 1)).astype(np.float16)

    if _cached_nc is None:
        _cached_nc = build_program()
    nc = _cached_nc

    in_maps = [{"xT": xT[b], "wqkv": wqkv, "wproj": wproj} for b in range(B)]
    kwargs = {}
    if trace:
        kwargs["trace"] = True
        if trace_kwargs:
            kwargs.update(trace_kwargs)
    try:
        res = run_bass_kernel_spmd(nc, in_maps, core_ids=list(range(B)), **kwargs)
    except Exception:
        # transient axon/PJRT hiccups happen; one retry
        res = run_bass_kernel_spmd(nc, in_maps, core_ids=list(range(B)), **kwargs)
    out = np.stack([np.asarray(r["y"], dtype=np.float32) for r in res.results],
                   axis=0)
    return out, res


def kernel(**inputs):
    out, _ = _run(inputs)
    return out
